# revision 21
# baseline (speedup 1.0000x reference)
"""Multi-head attention (QKV proj + SDPA + output proj) on 8 Trainium2 cores.

Sharding: tensor-parallel over heads. 16 heads / 8 cores = 2 heads per core.
Each core computes q/k/v for its 2 heads, SDPA, and a partial output
projection against its 128-column slice of proj_w. The host sums the 8
partial projections.

All data paths are bf16: the rms-rel-error gate (2e-2) rules out fp8
anywhere on the score/value path -- in softmax attention every
multiplicative quantization noise passes through to the output relative
error at full strength (the output magnitude shrinks by the same
sqrt(N_eff) averaging as the noise does), so fp8's >=2.5% per-element
noise alone would blow the budget (measured: e5m2 values + fp8 q/k =
7e-2 rms).

v3 changes vs the 373us baseline (from trace analysis):
  * QK head-pair concurrency. The PE runs row-tiled matmuls in different
    64-row groups concurrently, but only if they are adjacent in the
    issue queue. The baseline's per-head score psum tiles freed at
    different times (ACT vs DVE exp), so the scheduler's ready-queue
    separated the pairs. Now each chunk's scores live in ONE 6-bank psum
    tile [128, 2, 3, 512] consumed by exactly one ACT and one DVE
    instruction -> all six next-chunk QK matmuls become ready at the
    same instant and issue interleaved h0,h1 -> ~2x QK throughput.
  * exp split by query columns, not by head: DVE (Schraudolph, 1.27
    ns/col) takes cols [0:XQ), ACT (true exp, 0.82 ns/col) the rest, one
    instruction per engine per chunk. Row purity (a softmax row = one
    engine) is preserved per query column; per-row scale cancels in the
    softmax ratio.
  * normalization moved off the critical engines: denominator reciprocal
    via the fast approx DVE op (one [4,512] instr instead of 4 slow
    iterative quarters), broadcast AND the final multiply on the
    otherwise-idle GPSIMD engine (all-SBUF operands).
  * ~35 dummy matmuls at kernel start keep the PE HAM clock-gate warm
    through the x DMA so the projections start at 2.4GHz.

Softmax skips the max-subtraction: scores have std ~1 (scale=1/8, d=64,
unit-variance q/k), so exp() stays in bf16 range with huge margin.

Denominators come from a ones-column in the AV stationary (row 64 of the
av psum). The v bias and proj bias are linear post-terms (attn weights
sum to 1): bias = qkv_b[2048:] @ proj_w.T + proj_b, added on host.

PSUM (8 banks): sc [128,2,512] (2 banks x bufs=3) + av0 (1) + av1 (1).
"""

import numpy as np
import ml_dtypes

N_CORES = 8
SEQ = 4096
DMODEL = 1024
NHEADS = 16
DHEAD = 64
H_PER_CORE = NHEADS // N_CORES  # 2
CBLK = DMODEL // N_CORES  # 128 head-dim columns per core

IT = 512  # i (query) tile width
NI = SEQ // IT  # 8
JT = 128  # j (key) tile = psum partition dim
NJ = SEQ // JT  # 32
NCT = DMODEL // 128  # 8 contraction tiles for the projections
SCALE = DHEAD ** -0.5

# Schraudolph exp-as-bf16-bits constants (DVE path):
# u16 = round(s*A + B) where the u16 bit pattern IS bf16(exp(s*SCALE)).
SCHR_A = 128.0 * SCALE * 1.4426950408889634
SCHR_B = 128.0 * (127.0 + 0.043)

XQ = 256  # DVE query-column share per 512 (ACT takes the rest)

N_WARM = 20  # PE warmup dummy matmuls (cover the x DMA window)

_CACHE = {}


def _build_nc():
    import concourse.tile as tile
    from concourse import bacc, mybir

    bf16 = mybir.dt.bfloat16
    f16 = mybir.dt.float16
    f32 = mybir.dt.float32
    u16 = mybir.dt.uint16
    Exp = mybir.ActivationFunctionType.Exp
    Mult = mybir.AluOpType.mult
    Add = mybir.AluOpType.add

    nc = bacc.Bacc(
        "TRN2",
        target_bir_lowering=False,
        debug=False,
        enable_asserts=True,
        num_devices=N_CORES,
    )

    xT = nc.dram_tensor("xT", [DMODEL, SEQ], bf16, kind="ExternalInput").ap()
    wqk = nc.dram_tensor("wqk", [DMODEL, 256], bf16, kind="ExternalInput").ap()
    wv = nc.dram_tensor("wv", [DMODEL, CBLK], bf16, kind="ExternalInput").ap()
    pw = nc.dram_tensor("pw", [CBLK, DMODEL], bf16, kind="ExternalInput").ap()
    bqk = nc.dram_tensor("bqk", [128, 2], f32, kind="ExternalInput").ap()
    partialT = nc.dram_tensor(
        "partialT", [DMODEL, SEQ], f16, kind="ExternalOutput"
    ).ap()

    with tile.TileContext(nc) as tc:
        with (
            tc.tile_pool(name="weights", bufs=1) as wpool,
            tc.tile_pool(name="xtiles", bufs=NCT) as xpool,
            tc.tile_pool(name="qk", bufs=1) as qkpool,
            tc.tile_pool(name="vaug", bufs=NJ) as vpool,
            tc.tile_pool(name="exps", bufs=3) as epool,
            tc.tile_pool(name="attn", bufs=1) as apool,
            tc.tile_pool(name="norm", bufs=4) as npool,
            tc.tile_pool(name="stage", bufs=4) as stpool,
        ):
            # ---- PE warmup dummies: keep the HAM clock-gate warm through
            # the x DMA (no data deps beyond a DVE memset). ----
            warm_w = wpool.tile([1, 640], bf16)
            nc.vector.memset(warm_w[:], 1.0)
            pswarm = tc.tile_pool(name="pswarm", bufs=1, space="PSUM")
            pspool = pswarm.__enter__()
            warm_ps = pspool.tile([128, IT], f32)
            for _ in range(N_WARM):
                nc.tensor.matmul(
                    warm_ps[:], warm_w[0:1, 0:128], warm_w[0:1, 128:640],
                    start=True, stop=True,
                )
            pswarm.__exit__(None, None, None)

            # ---- load weights + x (wqk_c0 + x_c0 first so the first qk
            # matmuls start as soon as possible; wv/pw are needed later) ----
            wqk_t = []
            xt = []
            for c in range(NCT):
                wqk_c = wpool.tile([128, 256], bf16, name=f"wqk_c{c}")
                nc.sync.dma_start(wqk_c[:], wqk[c * 128 : (c + 1) * 128, :])
                wqk_t.append(wqk_c)
                x_c = xpool.tile([128, SEQ], bf16, name=f"x_c{c}", tag="xc")
                # split the 8MB x load across both hwdge queues (SP + ACT)
                eng = nc.sync if c % 2 == 0 else nc.scalar
                eng.dma_start(x_c[:], xT[c * 128 : (c + 1) * 128, :])
                xt.append(x_c)
            bqk_t = wpool.tile([128, 2], f32)
            nc.sync.dma_start(bqk_t[:], bqk[:])
            # tiny dummy exp: pulls the ~2.7us ACT table load off the
            # critical path (overlaps the x DMA).
            scratch = wpool.tile([1, 1], f32)
            nc.scalar.activation(scratch[:], bqk_t[0:1, 0:1], Exp)
            wv_t = []
            for c in range(NCT):
                wv_c = wpool.tile([128, CBLK], bf16, name=f"wv_c{c}")
                nc.sync.dma_start(wv_c[:], wv[c * 128 : (c + 1) * 128, :])
                wv_t.append(wv_c)
            pw_t = wpool.tile([128, DMODEL], bf16)
            nc.sync.dma_start(pw_t[:], pw[:])

            # vaug tiles + ones-column memsets up front (DVE idle during
            # the x DMA)
            vaug = []
            for j in range(NJ):
                va = vpool.tile([128, 130], bf16, name=f"vaug{j}", tag="vaug")
                nc.vector.memset(va[:, 64:65], 1.0)
                nc.vector.memset(va[:, 129:130], 1.0)
                vaug.append(va)

            # ---- QKV projections ----
            # qT/kT: [2*DHEAD=128, SEQ], stationary = w slices, moving = xT
            qT = qkpool.tile([128, SEQ], bf16)
            kT = qkpool.tile([128, SEQ], bf16)
            psqk = tc.tile_pool(name="psqk", bufs=1, space="PSUM")
            pspool = psqk.__enter__()
            for f, dest in ((0, qT), (1, kT)):
                ps = []
                for i in range(NI):
                    p = pspool.tile(
                        [128, IT], f32, name=f"qkps{f}_{i}", tag="qkps", bufs=8
                    )
                    ps.append(p)
                for c in range(NCT):
                    lhsT = wqk_t[c][:, f * 128 : (f + 1) * 128]
                    for i in range(NI):
                        nc.tensor.matmul(
                            ps[i][:],
                            lhsT,
                            xt[c][:, i * IT : (i + 1) * IT],
                            start=(c == 0),
                            stop=(c == NCT - 1),
                        )
                for i in range(NI):
                    if i % 2 == 0:
                        nc.vector.tensor_scalar_add(
                            dest[:, i * IT : (i + 1) * IT],
                            ps[i][:],
                            bqk_t[:, f : f + 1],
                        )
                    else:
                        nc.scalar.activation(
                            dest[:, i * IT : (i + 1) * IT],
                            ps[i][:],
                            mybir.ActivationFunctionType.Identity,
                            bias=bqk_t[:, f : f + 1],
                        )

            # v in natural layout [j, d] (+ ones column per head):
            # stationary = xT block, moving = wv.
            psqk.__exit__(None, None, None)
            psv = tc.tile_pool(name="psv", bufs=1, space="PSUM")
            pspool = psv.__enter__()
            for j in range(NJ):
                vp = pspool.tile([128, CBLK], f32, name=f"vps{j}", tag="vps", bufs=4)
                for c in range(NCT):
                    nc.tensor.matmul(
                        vp[:],
                        xt[c][:, j * JT : (j + 1) * JT],
                        wv_t[c][:],
                        start=(c == 0),
                        stop=(c == NCT - 1),
                    )
                va = vaug[j]
                if j % 2 == 0:
                    nc.vector.tensor_copy(va[:, 0:64], vp[:, 0:64])
                    nc.scalar.copy(va[:, 65:129], vp[:, 64:128])
                else:
                    nc.scalar.copy(va[:, 0:64], vp[:, 0:64])
                    nc.vector.tensor_copy(va[:, 65:129], vp[:, 64:128])

            # ---- attention ----
            # One 6-bank score tile per chunk [128, h, t, 512] so both
            # heads' next-chunk QK matmuls become ready simultaneously
            # (forces h0/h1 issue adjacency -> PE row-group concurrency).
            psv.__exit__(None, None, None)
            psattn = tc.tile_pool(name="psattn", bufs=1, space="PSUM")
            pspool = psattn.__enter__()
            attn_outT = apool.tile([128, SEQ], bf16)
            # per-i2 [2, IT] denominator/reciprocal tiles (engine partition
            # offsets must be 32-aligned, so each i2 starts at partition 0
            # of its own tile; DMA can address any partition)
            den_b = [
                npool.tile([2, IT], f32, name=f"den{b}", tag="den", bufs=8)
                for b in range(NI)
            ]
            rinv_b = [
                npool.tile([2, IT], f32, name=f"rinv{b}", tag="rinv", bufs=8)
                for b in range(NI)
            ]
            avs_t = {}
            rb_t = {}

            def normalize_prep(i2, h):
                # stage the reciprocal row at partition 0, broadcast to 64
                # rows on gpsimd
                r = i2 * 2 + h
                rt = npool.tile([1, IT], f32, name=f"rt{r}", tag="rt", bufs=4)
                nc.sync.dma_start(rt[:], rinv_b[i2][h : h + 1, :])
                rb = npool.tile([64, IT], f32, name=f"rb{r}", tag="rb", bufs=4)
                nc.gpsimd.partition_broadcast(rb[:], rt[:])
                rb_t[(i2, h)] = rb

            def normalize_mul(i2, h):
                # multiply avs rows by the broadcast reciprocal (gpsimd,
                # all-SBUF) -> attn_outT
                nc.gpsimd.tensor_mul(
                    attn_outT[h * 64 : (h + 1) * 64, i2 * IT : (i2 + 1) * IT],
                    avs_t[(i2, h)][0:64, :],
                    rb_t[(i2, h)][:],
                )

            def drain_av(i2, av_t, h):
                # av psum -> sbuf (row 64 = denominator -> DMA-gather into
                # the [4, IT] batch tiles for the batched reciprocal)
                avs = npool.tile(
                    [65, IT], f32, name=f"avs{h}_{i2}", tag="avs", bufs=4
                )
                if h == 0:
                    nc.scalar.copy(avs[:], av_t[h][0:65, :])
                else:
                    nc.vector.tensor_copy(avs[:], av_t[h][0:65, :])
                avs_t[(i2, h)] = avs
                nc.sync.dma_start(den_b[i2][h : h + 1, :], avs[64:65, :])

            # Software-pipelined chunk loop (chunk = one j-tile, sc bufs=3).
            # AV matmuls run at lag ~3 chunks behind their QK/exp: the
            # pending deque flushes its oldest entry once 3 are queued, so
            # an AV in the PE FIFO never head-of-line-blocks on an exp that
            # hasn't finished yet (exp latency is ~1.5 chunks).
            from collections import deque

            pend = deque()  # (i, av, e, j)

            def emit_av(p):
                _, av_p, e_p, j = p
                for h in range(2):
                    nc.tensor.matmul(
                        av_p[h][0:65, :],
                        vaug[j][:, h * 65 : h * 65 + 65],
                        e_p[:, h, :],
                        start=(j == 0),
                        stop=(j == NJ - 1),
                    )

            for i in range(NI):
                av = [
                    pspool.tile(
                        [128, IT], f32, name=f"av{h}_{i}", tag=f"av{h}", bufs=1
                    )
                    for h in range(2)
                ]
                for ci in range(NJ):
                    sc = pspool.tile(
                        [128, 2, IT], f32, name=f"sc_{i}_{ci}", tag="sc",
                        bufs=3,
                    )
                    for h in range(2):
                        nc.tensor.matmul(
                            sc[:, h, :],
                            kT[h * 64 : (h + 1) * 64, ci * JT : (ci + 1) * JT],
                            qT[h * 64 : (h + 1) * 64, i * IT : (i + 1) * IT],
                            start=True,
                            stop=True,
                            tile_position=(h * 64, 0),
                        )
                    if len(pend) >= 3:
                        p = pend.popleft()
                        emit_av(p)
                        if p[3] == NJ - 1:
                            # i-1's av is complete: drain both heads now so
                            # the slot frees before av(i)'s first AV flushes
                            drain_av(p[0], p[1], 0)
                            drain_av(p[0], p[1], 1)
                    # exp: one instruction per engine per chunk. DVE does
                    # query cols [0:XQ) via Schraudolph (u16 bits = bf16),
                    # ACT does [XQ:512) true exp. Row-pure per query.
                    e = epool.tile(
                        [128, 2, IT], bf16, name=f"e_{i}_{ci}", tag="e",
                        bufs=5,
                    )
                    nc.vector.tensor_scalar(
                        e[:, :, 0:XQ].bitcast(u16),
                        sc[:, :, 0:XQ],
                        SCHR_A,
                        SCHR_B,
                        Mult,
                        Add,
                    )
                    nc.scalar.activation(
                        e[:, :, XQ:IT],
                        sc[:, :, XQ:IT],
                        Exp,
                        scale=SCALE,
                    )
                    pend.append((i, av, e, ci))
                    if i >= 1:
                        p2 = i - 1
                        if ci == 6:
                            # fast approx reciprocal of i-1's 2 denominator
                            # rows (~18-bit; denominators are ~1e3-1e4)
                            nc.vector.reciprocal_approx_fast(
                                rinv_b[p2][:], den_b[p2][:]
                            )
                        elif ci == 10:
                            normalize_prep(p2, 0)
                        elif ci == 14:
                            normalize_prep(p2, 1)
                        elif ci == 18:
                            normalize_mul(p2, 0)
                        elif ci == 22:
                            normalize_mul(p2, 1)

            while pend:
                emit_av(pend.popleft())
            drain_av(NI - 1, av, 0)
            drain_av(NI - 1, av, 1)

            psattn.__exit__(None, None, None)
            psproj = tc.tile_pool(name="psproj", bufs=1, space="PSUM")
            pspool = psproj.__enter__()

            # ---- output projection (partial, this core's 128 hd cols) ----
            # tail normalization for i-tiles 6,7 interleaved with the first
            # projection tiles (PE idle >=3.4us would re-engage the HAM
            # half-clock gate).
            def tail_norm(step):
                if step == 0:
                    nc.vector.reciprocal_approx_fast(
                        rinv_b[NI - 1][:], den_b[NI - 1][:]
                    )
                elif step == 1:
                    normalize_prep(NI - 1, 0)
                    normalize_prep(NI - 1, 1)
                elif step == 2:
                    normalize_mul(NI - 1, 0)
                elif step == 3:
                    normalize_mul(NI - 1, 1)

            for i in range(NI):
                if i < 4:
                    tail_norm(i)
                for cc in range(NCT):
                    lhsT = pw_t[:, cc * 128 : (cc + 1) * 128]
                    pp = pspool.tile(
                        [128, IT], f32, name=f"pp{cc}_{i}", tag="pp", bufs=8
                    )
                    nc.tensor.matmul(
                        pp[:],
                        lhsT,
                        attn_outT[:, i * IT : (i + 1) * IT],
                        start=True,
                        stop=True,
                    )
                    st = stpool.tile(
                        [128, IT], f16, name=f"st{cc}_{i}", tag="st", bufs=8
                    )
                    if cc % 2 == 1:
                        nc.scalar.copy(st[:], pp[:])
                        out_eng = nc.scalar
                    else:
                        nc.vector.tensor_copy(st[:], pp[:])
                        out_eng = nc.sync
                    out_eng.dma_start(
                        partialT[
                            cc * 128 : (cc + 1) * 128, i * IT : (i + 1) * IT
                        ],
                        st[:],
                    )
            psproj.__exit__(None, None, None)

    nc.compile()
    return nc


def _get_nc():
    if "nc" not in _CACHE:
        _CACHE["nc"] = _build_nc()
    return _CACHE["nc"]


def build_in_maps(x, qkv_w, qkv_b, proj_w):
    bf16 = ml_dtypes.bfloat16

    x = np.asarray(x)
    qkv_w = np.asarray(qkv_w)
    qkv_b = np.asarray(qkv_b)
    proj_w = np.asarray(proj_w)

    x2d = np.ascontiguousarray(x.reshape(SEQ, DMODEL).T).astype(bf16)  # [1024, 4096]

    in_maps = []
    for c in range(N_CORES):
        lo, hi = c * CBLK, (c + 1) * CBLK
        wq_c = qkv_w[lo:hi, :]  # [128, 1024]
        wk_c = qkv_w[DMODEL + lo : DMODEL + hi, :]
        wv_c = qkv_w[2 * DMODEL + lo : 2 * DMODEL + hi, :]
        in_maps.append(
            {
                "xT": x2d,
                "wqk": np.ascontiguousarray(
                    np.concatenate([wq_c.T, wk_c.T], axis=1)
                ).astype(bf16),
                "wv": np.ascontiguousarray(wv_c.T).astype(bf16),
                "pw": np.ascontiguousarray(proj_w[:, lo:hi].T).astype(bf16),
                "bqk": np.ascontiguousarray(
                    np.stack(
                        [qkv_b[lo:hi], qkv_b[DMODEL + lo : DMODEL + hi]], axis=1
                    )
                ).astype(np.float32),
            }
        )
    return in_maps


def kernel(x, qkv_w, qkv_b, proj_w, proj_b):
    from concourse.bass_utils import run_bass_kernel_spmd

    nc = _get_nc()
    in_maps = build_in_maps(x, qkv_w, qkv_b, proj_w)
    res = run_bass_kernel_spmd(nc, in_maps, core_ids=list(range(N_CORES)))

    acc = np.zeros((DMODEL, SEQ), dtype=np.float32)
    for c in range(N_CORES):
        acc += res.results[c]["partialT"].astype(np.float32)

    # host-side linear bias terms: proj bias + v-bias routed through proj
    bias = qkv_b[2 * DMODEL :].astype(np.float32) @ proj_w.T.astype(
        np.float32
    ) + proj_b.astype(np.float32)
    out = acc.T + bias[None, :]
    return out.reshape(1, SEQ, DMODEL).astype(np.float32)


# revision 23
# speedup vs baseline: 1.1354x; 1.1354x over previous
"""Multi-head attention (QKV proj + SDPA + output proj) on 8 Trainium2 cores.

Sharding: tensor-parallel over heads. 16 heads / 8 cores = 2 heads per core.
Each core computes q/k/v for its 2 heads, SDPA, and a partial output
projection against its 128-column slice of proj_w. The host sums the 8
partial projections.

All data paths are bf16: the rms-rel-error gate (2e-2) rules out fp8
anywhere on the score/value path -- in softmax attention every
multiplicative quantization noise passes through to the output relative
error at full strength (the output magnitude shrinks by the same
sqrt(N_eff) averaging as the noise does), so fp8's >=2.5% per-element
noise alone would blow the budget (measured: e5m2 values + fp8 q/k =
7e-2 rms).

v3 changes vs the 373us baseline (from trace analysis):
  * QK head-pair concurrency. The PE runs row-tiled matmuls in different
    64-row groups concurrently, but only if they are adjacent in the
    issue queue. The baseline's per-head score psum tiles freed at
    different times (ACT vs DVE exp), so the scheduler's ready-queue
    separated the pairs. Now each chunk's scores live in ONE 6-bank psum
    tile [128, 2, 3, 512] consumed by exactly one ACT and one DVE
    instruction -> all six next-chunk QK matmuls become ready at the
    same instant and issue interleaved h0,h1 -> ~2x QK throughput.
  * exp split by query columns, not by head: DVE (Schraudolph, 1.27
    ns/col) takes cols [0:XQ), ACT (true exp, 0.82 ns/col) the rest, one
    instruction per engine per chunk. Row purity (a softmax row = one
    engine) is preserved per query column; per-row scale cancels in the
    softmax ratio.
  * normalization moved off the critical engines: denominator reciprocal
    via the fast approx DVE op (one [4,512] instr instead of 4 slow
    iterative quarters), broadcast AND the final multiply on the
    otherwise-idle GPSIMD engine (all-SBUF operands).
  * ~35 dummy matmuls at kernel start keep the PE HAM clock-gate warm
    through the x DMA so the projections start at 2.4GHz.

Softmax skips the max-subtraction: scores have std ~1 (scale=1/8, d=64,
unit-variance q/k), so exp() stays in bf16 range with huge margin.

Denominators come from a ones-column in the AV stationary (row 64 of the
av psum). The v bias and proj bias are linear post-terms (attn weights
sum to 1): bias = qkv_b[2048:] @ proj_w.T + proj_b, added on host.

PSUM (8 banks): sc [128,2,512] (2 banks x bufs=3) + av0 (1) + av1 (1).
"""

import numpy as np
import ml_dtypes

N_CORES = 8
SEQ = 4096
DMODEL = 1024
NHEADS = 16
DHEAD = 64
H_PER_CORE = NHEADS // N_CORES  # 2
CBLK = DMODEL // N_CORES  # 128 head-dim columns per core

IT = 512  # i (query) tile width
NI = SEQ // IT  # 8
JT = 128  # j (key) tile = psum partition dim
NJ = SEQ // JT  # 32
NCT = DMODEL // 128  # 8 contraction tiles for the projections
SCALE = DHEAD ** -0.5

# Schraudolph exp-as-bf16-bits constants (DVE path):
# u16 = round(s*A + B) where the u16 bit pattern IS bf16(exp(s*SCALE)).
SCHR_A = 128.0 * SCALE * 1.4426950408889634
SCHR_B = 128.0 * (127.0 + 0.043)

XQ = 132  # DVE query-column share per 512 (ACT takes the rest)

N_WARM = 20  # PE warmup dummy matmuls (cover the x DMA window)

_CACHE = {}


def _build_nc():
    import concourse.tile as tile
    from concourse import bacc, mybir

    bf16 = mybir.dt.bfloat16
    f16 = mybir.dt.float16
    f32 = mybir.dt.float32
    u16 = mybir.dt.uint16
    Exp = mybir.ActivationFunctionType.Exp
    Mult = mybir.AluOpType.mult
    Add = mybir.AluOpType.add

    nc = bacc.Bacc(
        "TRN2",
        target_bir_lowering=False,
        debug=False,
        enable_asserts=True,
        num_devices=N_CORES,
    )

    xT = nc.dram_tensor("xT", [DMODEL, SEQ], bf16, kind="ExternalInput").ap()
    wqk = nc.dram_tensor("wqk", [DMODEL, 256], bf16, kind="ExternalInput").ap()
    wv = nc.dram_tensor("wv", [DMODEL, CBLK], bf16, kind="ExternalInput").ap()
    pw = nc.dram_tensor("pw", [CBLK, DMODEL], bf16, kind="ExternalInput").ap()
    bqk = nc.dram_tensor("bqk", [128, 2], f32, kind="ExternalInput").ap()
    partialT = nc.dram_tensor(
        "partialT", [DMODEL, SEQ], f16, kind="ExternalOutput"
    ).ap()

    with tile.TileContext(nc) as tc:
        with (
            tc.tile_pool(name="weights", bufs=1) as wpool,
            tc.tile_pool(name="xtiles", bufs=NCT) as xpool,
            tc.tile_pool(name="qk", bufs=1) as qkpool,
            tc.tile_pool(name="vaug", bufs=NJ) as vpool,
            tc.tile_pool(name="exps", bufs=3) as epool,
            tc.tile_pool(name="attn", bufs=1) as apool,
            tc.tile_pool(name="norm", bufs=4) as npool,
            tc.tile_pool(name="stage", bufs=4) as stpool,
        ):
            # ---- PE warmup dummies: keep the HAM clock-gate warm through
            # the x DMA (no data deps beyond a DVE memset). ----
            warm_w = wpool.tile([1, 640], bf16)
            nc.vector.memset(warm_w[:], 1.0)
            pswarm = tc.tile_pool(name="pswarm", bufs=1, space="PSUM")
            pspool = pswarm.__enter__()
            warm_ps = pspool.tile([128, IT], f32)
            for _ in range(N_WARM):
                nc.tensor.matmul(
                    warm_ps[:], warm_w[0:1, 0:128], warm_w[0:1, 128:640],
                    start=True, stop=True,
                )
            pswarm.__exit__(None, None, None)

            # ---- load weights + x (wqk_c0 + x_c0 first so the first qk
            # matmuls start as soon as possible; wv/pw are needed later) ----
            wqk_t = []
            xt = []
            for c in range(NCT):
                wqk_c = wpool.tile([128, 256], bf16, name=f"wqk_c{c}")
                nc.sync.dma_start(wqk_c[:], wqk[c * 128 : (c + 1) * 128, :])
                wqk_t.append(wqk_c)
                x_c = xpool.tile([128, SEQ], bf16, name=f"x_c{c}", tag="xc")
                # split the 8MB x load across both hwdge queues (SP + ACT)
                eng = nc.sync if c % 2 == 0 else nc.scalar
                eng.dma_start(x_c[:], xT[c * 128 : (c + 1) * 128, :])
                xt.append(x_c)
            bqk_t = wpool.tile([128, 2], f32)
            nc.sync.dma_start(bqk_t[:], bqk[:])
            # tiny dummy exp: pulls the ~2.7us ACT table load off the
            # critical path (overlaps the x DMA).
            scratch = wpool.tile([1, 1], f32)
            nc.scalar.activation(scratch[:], bqk_t[0:1, 0:1], Exp)
            wv_t = []
            for c in range(NCT):
                wv_c = wpool.tile([128, CBLK], bf16, name=f"wv_c{c}")
                nc.sync.dma_start(wv_c[:], wv[c * 128 : (c + 1) * 128, :])
                wv_t.append(wv_c)
            pw_t = wpool.tile([128, DMODEL], bf16)
            nc.sync.dma_start(pw_t[:], pw[:])

            # vaug tiles + ones-column memsets up front (DVE idle during
            # the x DMA)
            vaug = []
            for j in range(NJ):
                va = vpool.tile([128, 130], bf16, name=f"vaug{j}", tag="vaug")
                nc.vector.memset(va[:, 64:65], 1.0)
                nc.vector.memset(va[:, 129:130], 1.0)
                vaug.append(va)

            # ---- QKV projections ----
            # qT/kT: [2*DHEAD=128, SEQ], stationary = w slices, moving = xT
            qT = qkpool.tile([128, SEQ], bf16)
            kT = qkpool.tile([128, SEQ], bf16)
            psqk = tc.tile_pool(name="psqk", bufs=1, space="PSUM")
            pspool = psqk.__enter__()
            for f, dest in ((0, qT), (1, kT)):
                ps = []
                for i in range(NI):
                    p = pspool.tile(
                        [128, IT], f32, name=f"qkps{f}_{i}", tag="qkps", bufs=8
                    )
                    ps.append(p)
                for c in range(NCT):
                    lhsT = wqk_t[c][:, f * 128 : (f + 1) * 128]
                    for i in range(NI):
                        nc.tensor.matmul(
                            ps[i][:],
                            lhsT,
                            xt[c][:, i * IT : (i + 1) * IT],
                            start=(c == 0),
                            stop=(c == NCT - 1),
                        )
                for i in range(NI):
                    if i % 2 == 0:
                        nc.vector.tensor_scalar_add(
                            dest[:, i * IT : (i + 1) * IT],
                            ps[i][:],
                            bqk_t[:, f : f + 1],
                        )
                    else:
                        nc.scalar.activation(
                            dest[:, i * IT : (i + 1) * IT],
                            ps[i][:],
                            mybir.ActivationFunctionType.Identity,
                            bias=bqk_t[:, f : f + 1],
                        )

            # v in natural layout [j, d] (+ ones column per head):
            # stationary = xT block, moving = wv.
            psqk.__exit__(None, None, None)
            psv = tc.tile_pool(name="psv", bufs=1, space="PSUM")
            pspool = psv.__enter__()
            for j in range(NJ):
                vp = pspool.tile([128, CBLK], f32, name=f"vps{j}", tag="vps", bufs=4)
                for c in range(NCT):
                    nc.tensor.matmul(
                        vp[:],
                        xt[c][:, j * JT : (j + 1) * JT],
                        wv_t[c][:],
                        start=(c == 0),
                        stop=(c == NCT - 1),
                    )
                va = vaug[j]
                if j % 2 == 0:
                    nc.vector.tensor_copy(va[:, 0:64], vp[:, 0:64])
                    nc.scalar.copy(va[:, 65:129], vp[:, 64:128])
                else:
                    nc.scalar.copy(va[:, 0:64], vp[:, 0:64])
                    nc.vector.tensor_copy(va[:, 65:129], vp[:, 64:128])

            # ---- attention ----
            # One 6-bank score tile per chunk [128, h, t, 512] so both
            # heads' next-chunk QK matmuls become ready simultaneously
            # (forces h0/h1 issue adjacency -> PE row-group concurrency).
            psv.__exit__(None, None, None)
            psattn = tc.tile_pool(name="psattn", bufs=1, space="PSUM")
            pspool = psattn.__enter__()
            attn_outT = apool.tile([128, SEQ], bf16)
            # per-i2 [2, IT] denominator/reciprocal tiles (engine partition
            # offsets must be 32-aligned, so each i2 starts at partition 0
            # of its own tile; DMA can address any partition)
            den_b = [
                npool.tile([2, IT], f32, name=f"den{b}", tag="den", bufs=8)
                for b in range(NI)
            ]
            rinv_b = [
                npool.tile([2, IT], f32, name=f"rinv{b}", tag="rinv", bufs=8)
                for b in range(NI)
            ]
            avs_t = {}
            rb_t = {}

            def normalize_prep(i2, h):
                # stage the reciprocal row at partition 0, broadcast to 64
                # rows on gpsimd
                r = i2 * 2 + h
                rt = npool.tile([1, IT], f32, name=f"rt{r}", tag="rt", bufs=4)
                nc.sync.dma_start(rt[:], rinv_b[i2][h : h + 1, :])
                rb = npool.tile([64, IT], f32, name=f"rb{r}", tag="rb", bufs=4)
                nc.gpsimd.partition_broadcast(rb[:], rt[:])
                rb_t[(i2, h)] = rb

            def normalize_mul(i2, h):
                # multiply avs rows by the broadcast reciprocal (gpsimd,
                # all-SBUF) -> attn_outT
                nc.gpsimd.tensor_mul(
                    attn_outT[h * 64 : (h + 1) * 64, i2 * IT : (i2 + 1) * IT],
                    avs_t[(i2, h)][0:64, :],
                    rb_t[(i2, h)][:],
                )

            def drain_av(i2, av_t, h):
                # av psum -> sbuf (row 64 = denominator -> DMA-gather into
                # the [4, IT] batch tiles for the batched reciprocal)
                avs = npool.tile(
                    [65, IT], f32, name=f"avs{h}_{i2}", tag="avs", bufs=4
                )
                if h == 0:
                    nc.scalar.copy(avs[:], av_t[h][0:65, :])
                else:
                    nc.vector.tensor_copy(avs[:], av_t[h][0:65, :])
                avs_t[(i2, h)] = avs
                nc.sync.dma_start(den_b[i2][h : h + 1, :], avs[64:65, :])

            # Software-pipelined chunk loop (chunk = one j-tile, sc bufs=3
            # -> QK(n+3) waits only exp(n): ~3 chunks of slack keeps the
            # PE stream dense while the exps run). AV matmuls for chunk n
            # are emitted after chunk n+1's QK.
            pending = None  # (i, av, e, j)

            def emit_av(p):
                _, av_p, e_p, j = p
                for h in range(2):
                    nc.tensor.matmul(
                        av_p[h][0:65, :],
                        vaug[j][:, h * 65 : h * 65 + 65],
                        e_p[:, h, :],
                        start=(j == 0),
                        stop=(j == NJ - 1),
                    )

            av_prev = None
            for i in range(NI):
                av = [
                    pspool.tile(
                        [128, IT], f32, name=f"av{h}_{i}", tag=f"av{h}", bufs=1
                    )
                    for h in range(2)
                ]
                for ci in range(NJ):
                    sc = pspool.tile(
                        [128, 2, IT], f32, name=f"sc_{i}_{ci}", tag="sc",
                        bufs=3,
                    )
                    for h in range(2):
                        nc.tensor.matmul(
                            sc[:, h, :],
                            kT[h * 64 : (h + 1) * 64, ci * JT : (ci + 1) * JT],
                            qT[h * 64 : (h + 1) * 64, i * IT : (i + 1) * IT],
                            start=True,
                            stop=True,
                            tile_position=(h * 64, 0),
                        )
                    flushed_prev = None
                    if pending is not None:
                        emit_av(pending)
                        if pending[0] != i:  # just flushed i-1's last chunk
                            flushed_prev = pending[1]
                    # exp: one instruction per engine per chunk. DVE does
                    # query cols [0:XQ) via Schraudolph (u16 bits = bf16),
                    # ACT does [XQ:512) true exp. Row-pure per query.
                    e = epool.tile(
                        [128, 2, IT], bf16, name=f"e_{i}_{ci}", tag="e",
                        bufs=4,
                    )
                    nc.vector.tensor_scalar(
                        e[:, :, 0:XQ].bitcast(u16),
                        sc[:, :, 0:XQ],
                        SCHR_A,
                        SCHR_B,
                        Mult,
                        Add,
                    )
                    nc.scalar.activation(
                        e[:, :, XQ:IT],
                        sc[:, :, XQ:IT],
                        Exp,
                        scale=SCALE,
                    )
                    pending = (i, av, e, ci)

                    # Post-processing of earlier i-tiles, spread across
                    # chunk slots so no engine gets a burst.
                    if flushed_prev is not None:
                        av_prev = flushed_prev
                    if av_prev is not None:
                        if ci == 0:
                            drain_av(i - 1, av_prev, 0)
                        elif ci == 1:
                            drain_av(i - 1, av_prev, 1)
                    if i >= 1:
                        p2 = i - 1
                        if ci == 4:
                            # fast approx reciprocal of i-1's 2 denominator
                            # rows (~18-bit; denominators are ~1e3-1e4)
                            nc.vector.reciprocal_approx_fast(
                                rinv_b[p2][:], den_b[p2][:]
                            )
                        elif ci == 8:
                            normalize_prep(p2, 0)
                        elif ci == 12:
                            normalize_prep(p2, 1)
                        elif ci == 16:
                            normalize_mul(p2, 0)
                        elif ci == 20:
                            normalize_mul(p2, 1)

            emit_av(pending)
            drain_av(NI - 1, pending[1], 0)
            drain_av(NI - 1, pending[1], 1)

            psattn.__exit__(None, None, None)
            psproj = tc.tile_pool(name="psproj", bufs=1, space="PSUM")
            pspool = psproj.__enter__()

            # ---- output projection (partial, this core's 128 hd cols) ----
            # tail normalization for i-tiles 6,7 interleaved with the first
            # projection tiles (PE idle >=3.4us would re-engage the HAM
            # half-clock gate).
            def tail_norm(step):
                if step == 0:
                    nc.vector.reciprocal_approx_fast(
                        rinv_b[NI - 1][:], den_b[NI - 1][:]
                    )
                elif step == 1:
                    normalize_prep(NI - 1, 0)
                    normalize_prep(NI - 1, 1)
                elif step == 2:
                    normalize_mul(NI - 1, 0)
                elif step == 3:
                    normalize_mul(NI - 1, 1)

            for i in range(NI):
                if i < 4:
                    tail_norm(i)
                for cc in range(NCT):
                    lhsT = pw_t[:, cc * 128 : (cc + 1) * 128]
                    pp = pspool.tile(
                        [128, IT], f32, name=f"pp{cc}_{i}", tag="pp", bufs=8
                    )
                    nc.tensor.matmul(
                        pp[:],
                        lhsT,
                        attn_outT[:, i * IT : (i + 1) * IT],
                        start=True,
                        stop=True,
                    )
                    st = stpool.tile(
                        [128, IT], f16, name=f"st{cc}_{i}", tag="st", bufs=8
                    )
                    if cc % 2 == 1:
                        nc.scalar.copy(st[:], pp[:])
                        out_eng = nc.scalar
                    else:
                        nc.vector.tensor_copy(st[:], pp[:])
                        out_eng = nc.sync
                    out_eng.dma_start(
                        partialT[
                            cc * 128 : (cc + 1) * 128, i * IT : (i + 1) * IT
                        ],
                        st[:],
                    )
            psproj.__exit__(None, None, None)

    nc.compile()
    return nc


def _get_nc():
    if "nc" not in _CACHE:
        _CACHE["nc"] = _build_nc()
    return _CACHE["nc"]


def build_in_maps(x, qkv_w, qkv_b, proj_w):
    bf16 = ml_dtypes.bfloat16

    x = np.asarray(x)
    qkv_w = np.asarray(qkv_w)
    qkv_b = np.asarray(qkv_b)
    proj_w = np.asarray(proj_w)

    x2d = np.ascontiguousarray(x.reshape(SEQ, DMODEL).T).astype(bf16)  # [1024, 4096]

    in_maps = []
    for c in range(N_CORES):
        lo, hi = c * CBLK, (c + 1) * CBLK
        wq_c = qkv_w[lo:hi, :]  # [128, 1024]
        wk_c = qkv_w[DMODEL + lo : DMODEL + hi, :]
        wv_c = qkv_w[2 * DMODEL + lo : 2 * DMODEL + hi, :]
        in_maps.append(
            {
                "xT": x2d,
                "wqk": np.ascontiguousarray(
                    np.concatenate([wq_c.T, wk_c.T], axis=1)
                ).astype(bf16),
                "wv": np.ascontiguousarray(wv_c.T).astype(bf16),
                "pw": np.ascontiguousarray(proj_w[:, lo:hi].T).astype(bf16),
                "bqk": np.ascontiguousarray(
                    np.stack(
                        [qkv_b[lo:hi], qkv_b[DMODEL + lo : DMODEL + hi]], axis=1
                    )
                ).astype(np.float32),
            }
        )
    return in_maps


def kernel(x, qkv_w, qkv_b, proj_w, proj_b):
    from concourse.bass_utils import run_bass_kernel_spmd

    nc = _get_nc()
    in_maps = build_in_maps(x, qkv_w, qkv_b, proj_w)
    res = run_bass_kernel_spmd(nc, in_maps, core_ids=list(range(N_CORES)))

    acc = np.zeros((DMODEL, SEQ), dtype=np.float32)
    for c in range(N_CORES):
        acc += res.results[c]["partialT"].astype(np.float32)

    # host-side linear bias terms: proj bias + v-bias routed through proj
    bias = qkv_b[2 * DMODEL :].astype(np.float32) @ proj_w.T.astype(
        np.float32
    ) + proj_b.astype(np.float32)
    out = acc.T + bias[None, :]
    return out.reshape(1, SEQ, DMODEL).astype(np.float32)
